# revision 15
# baseline (speedup 1.0000x reference)
"""DeeperGCN (4-layer GENConv, softmax aggregation) on 8 Trainium2 NeuronCores.

Strategy (dst-sharded graph parallelism):
  - Nodes are partitioned across the 8 cores (balanced by in-degree); each core
    owns the segment-softmax aggregation + MLP for its nodes.
  - Per layer, each core computes node tables P = exp(t*(relu(z)+eps) - 8) and
    R = (relu(z)+eps)*P for its own nodes (the per-segment max subtraction of
    the reference cancels algebraically; a constant offset of 8 keeps exp in
    range), AllGathers the bf16 [N,128] P|R table to every core's DRAM, then
    gathers per-edge rows with dma_gather and reduces them per destination
    with one-hot matmuls on the TensorEngine (32-dst windows, PSUM f32
    accumulation).  agg = sum(R_src)/sum(P_src) reproduces the reference's
    softmax-weighted message mean.
  - MLP / LayerNorms / residuals are dense per-node ops done tile-by-tile
    (128-node tiles), batched across tiles where possible.

kernel(**inputs) takes the FULL reference inputs and returns the FULL
[30000, 40] log-softmax output.
"""

import numpy as np
import ml_dtypes

N = 30000
E = 960000
F_IN = 128
H = 64
C = 40
L = 4
EPS = 1e-7
M_OFF = 8.0        # constant exp offset (replaces per-segment max; cancels)

NC_ = 8            # cores
TILES = 30         # 128-node tiles per core
NPC = TILES * 128  # padded nodes per core (3840)
NPAD = NC_ * NPC   # 30720 (< int16 max)
WPT = 4            # 32-dst windows per tile
WIN = 32
NWIN = TILES * WPT  # 120 windows per core
GROUP = 4          # node tiles per PSUM bank group

_CACHE = {}
LAST_RESULTS = None  # BassKernelResults of the most recent run (for test.py)


# --------------------------------------------------------------------------
# Host-side graph preprocessing (pure index manipulation, no float math)
# --------------------------------------------------------------------------

def _preprocess(edge_index):
    import heapq

    src = np.asarray(edge_index[0], dtype=np.int64)
    dst = np.asarray(edge_index[1], dtype=np.int64)
    deg = np.bincount(dst, minlength=N)

    # LPT-assign nodes to 8*120 windows (capacity 32), balancing edge load.
    order = np.argsort(-deg, kind="stable")
    nwin_g = NC_ * NWIN
    heap = [(0, w) for w in range(nwin_g)]
    heapq.heapify(heap)
    cap = np.zeros(nwin_g, np.int64)
    node_win = np.empty(N, np.int64)
    node_slot = np.empty(N, np.int64)
    for n in order:
        load, w = heapq.heappop(heap)
        node_win[n] = w
        node_slot[n] = cap[w]
        cap[w] += 1
        if cap[w] < WIN:
            heapq.heappush(heap, (load + int(deg[n]), w))

    wload = np.zeros(nwin_g, np.int64)
    np.add.at(wload, node_win[dst], 1)
    node_core = node_win // NWIN

    # Per core, order windows by load (desc) -> position, so the per-position
    # max across cores (which fixes the shared batch schedule) stays tight.
    pos_of_win = np.empty(nwin_g, np.int64)
    for c in range(NC_):
        wins = np.arange(c * NWIN, (c + 1) * NWIN)
        owins = wins[np.argsort(-wload[wins], kind="stable")]
        pos_of_win[owins] = np.arange(NWIN)

    loads = np.zeros((NC_, NWIN), np.int64)
    for c in range(NC_):
        wins = np.arange(c * NWIN, (c + 1) * NWIN)
        loads[c, pos_of_win[wins]] = wload[wins]
    B = np.maximum(1, -(-loads.max(axis=0) // 128)).astype(np.int64)  # [120]

    node_pos = pos_of_win[node_win]
    node_row = node_pos * WIN + node_slot          # row within core [0, 3840)
    table_row = node_core * NPC + node_row         # global table row (<30720)

    Bt = B.reshape(TILES, WPT)
    n_tile = Bt.sum(axis=1) * 128                  # gather idx slots per tile
    tile_col_base = np.zeros(TILES, np.int64)
    tile_col_base[1:] = np.cumsum(n_tile // 16)[:-1]
    tile_batch_base = np.zeros(TILES, np.int64)
    tile_batch_base[1:] = np.cumsum(Bt.sum(axis=1))[:-1]
    win_off = np.zeros((TILES, WPT), np.int64)     # idx-slot offset in tile
    win_off[:, 1:] = np.cumsum(Bt * 128, axis=1)[:, :-1]
    S_tot = int(n_tile.sum())
    TB = int(Bt.sum())

    # Edge placement
    e_core = node_core[dst]
    e_pos = node_pos[dst]
    key = e_core * NWIN + e_pos
    sort_i = np.argsort(key, kind="stable")
    ks = key[sort_i]
    grp_start = np.searchsorted(ks, np.arange(nwin_g))
    rank = np.arange(E) - grp_start[ks]
    t_of = (ks % NWIN) // WPT
    w_of = (ks % NWIN) % WPT
    assert (rank < Bt[t_of, w_of] * 128).all()
    i_tile = win_off[t_of, w_of] + rank            # slot within tile stream
    c_of = ks // NWIN

    idx_slab = np.zeros((NC_, 16, S_tot // 16), np.int16)
    srcrow = table_row[src[sort_i]].astype(np.int16)
    col = tile_col_base[t_of] + i_tile // 16
    idx_slab[c_of, i_tile % 16, col] = srcrow
    idx_slab = np.tile(idx_slab, (1, 8, 1))        # replicate to 128 parts

    oneh = np.zeros((NC_, 128, TB * WIN), ml_dtypes.bfloat16)
    gb = tile_batch_base[t_of] + i_tile // 128
    slotd = node_slot[dst[sort_i]]
    oneh[c_of, i_tile % 128, gb * WIN + slotd] = 1.0

    # batch schedule (shared across cores): per tile, list of (j, w, st, sp)
    batches = []
    for t in range(TILES):
        bl = []
        j = 0
        for w in range(WPT):
            for k in range(Bt[t, w]):
                bl.append((j, w, k == 0, k == Bt[t, w] - 1))
                j += 1
        batches.append(bl)

    node_of = np.full((NC_, NPC), -1, np.int64)
    node_of[node_core, node_row] = np.arange(N)

    return dict(
        idx_slab=idx_slab, oneh=oneh, batches=batches,
        n_tile=n_tile, tile_col_base=tile_col_base,
        tile_batch_base=tile_batch_base, S_tot=S_tot, TB=TB,
        node_of=node_of, maxb=int(Bt.sum(axis=1).max()),
    )


# --------------------------------------------------------------------------
# Bass kernel builder
# --------------------------------------------------------------------------

def _build(meta, triv, n_swdge_queues=1, stage="full", nlayers=L):
    import concourse.bass as bass
    import concourse.bacc as bacc
    import concourse.tile as tile
    import concourse.mybir as mybir
    from concourse.masks import make_identity

    f32 = mybir.dt.float32
    bf16 = mybir.dt.bfloat16
    i16 = mybir.dt.int16
    AF = mybir.ActivationFunctionType
    OP = mybir.AluOpType
    AX = mybir.AxisListType

    batches = meta["batches"]
    n_tile = meta["n_tile"]
    tcb = meta["tile_col_base"]
    tbb = meta["tile_batch_base"]
    S_tot = meta["S_tot"]
    TB = meta["TB"]
    MAXB = meta["maxb"]
    t_triv = triv["t"]
    ln1_triv = triv["ln1"]
    b1_triv = triv["b1"]
    b2_triv = triv["b2"]
    encb_triv = triv["encb"]
    linb_triv = triv["linb"]

    nc = bacc.Bacc("TRN2", target_bir_lowering=False, debug=False,
                   enable_asserts=False, num_devices=NC_,
                   num_swdge_queues=n_swdge_queues)

    # ---- I/O ----
    x_d = nc.dram_tensor("x_sh", [NPC, F_IN], f32, kind="ExternalInput")
    idx_d = nc.dram_tensor("idxs", [128, S_tot // 16], i16, kind="ExternalInput")
    oneh_d = nc.dram_tensor("oneh", [128, TB * WIN], bf16, kind="ExternalInput")
    encw_d = nc.dram_tensor("encW", [F_IN, H], f32, kind="ExternalInput")
    encb_d = nc.dram_tensor("encb", [1, H], f32, kind="ExternalInput")
    t_d = nc.dram_tensor("tvec", [1, L], f32, kind="ExternalInput")
    w1_d = nc.dram_tensor("w1", [H, L, 2 * H], f32, kind="ExternalInput")
    b1_d = nc.dram_tensor("b1r", [1, L, 2 * H], f32, kind="ExternalInput")
    ln1g_d = nc.dram_tensor("ln1g", [1, L, 2 * H], f32, kind="ExternalInput")
    ln1b_d = nc.dram_tensor("ln1b", [1, L, 2 * H], f32, kind="ExternalInput")
    w2_d = nc.dram_tensor("w2", [2 * H, L, H], f32, kind="ExternalInput")
    b2_d = nc.dram_tensor("b2r", [1, L, H], f32, kind="ExternalInput")
    ngrep_d = nc.dram_tensor("ngrep", [1, L, H], f32, kind="ExternalInput")
    nbrep_d = nc.dram_tensor("nbrep", [1, L, H], f32, kind="ExternalInput")
    linw_d = nc.dram_tensor("linW", [H, C], f32, kind="ExternalInput")
    linb_d = nc.dram_tensor("linb", [1, C], f32, kind="ExternalInput")
    out_d = nc.dram_tensor("out", [NPC, C], f32, kind="ExternalOutput")

    NF = TILES * H  # 1920 free elems for full-core node slabs

    def pb(ap, p=128):
        """[1, ...] AP -> [p, F] with 0-stride partition broadcast."""
        b = ap.partition_broadcast(p)
        names = " ".join(f"d{i}" for i in range(len(b.shape) - 1))
        return b.rearrange(f"p {names} -> p ({names})")

    with tile.TileContext(nc) as tc:
        with (
            tc.tile_pool(name="const", bufs=1) as cp,
            tc.tile_pool(name="slab", bufs=1) as sp,
            tc.tile_pool(name="gather", bufs=3) as gp,
            tc.tile_pool(name="work", bufs=3) as wp,
            tc.tile_pool(name="grp", bufs=2) as grp_pool,
            tc.tile_pool(name="ps", bufs=2, space="PSUM") as pp,
            tc.tile_pool(name="dram", bufs=1, space="DRAM") as dp,
        ):
            # ---- constants into SBUF ----
            oneh_sb = cp.tile([128, TB * WIN], bf16, tag="oneh")
            nc.sync.dma_start(oneh_sb[:, :], oneh_d.ap())
            idx_sb = cp.tile([128, S_tot // 16], i16, tag="idx")
            nc.sync.dma_start(idx_sb[:, :], idx_d.ap())
            encw_sb = cp.tile([F_IN, H], f32, tag="encw")
            nc.sync.dma_start(encw_sb[:, :], encw_d.ap())
            w1_sb = cp.tile([H, L * 2 * H], f32, tag="w1")
            nc.sync.dma_start(
                w1_sb[:, :].rearrange("p (l m) -> p l m", l=L), w1_d.ap())
            w2_sb = cp.tile([2 * H, L * H], f32, tag="w2")
            nc.sync.dma_start(
                w2_sb[:, :].rearrange("p (l m) -> p l m", l=L), w2_d.ap())
            linw_sb = cp.tile([H, C], f32, tag="linw")
            nc.sync.dma_start(linw_sb[:, :], linw_d.ap())
            ngrep_sb = cp.tile([1, L * H], f32, tag="ngrep")
            nc.sync.dma_start(
                ngrep_sb[:, :].rearrange("p (l m) -> p l m", l=L), ngrep_d.ap())
            nbrep_sb = cp.tile([1, L * H], f32, tag="nbrep")
            nc.sync.dma_start(
                nbrep_sb[:, :].rearrange("p (l m) -> p l m", l=L), nbrep_d.ap())
            ln1g_sb = cp.tile([1, L * 2 * H], f32, tag="ln1g")
            nc.sync.dma_start(
                ln1g_sb[:, :].rearrange("p (l m) -> p l m", l=L), ln1g_d.ap())
            ln1b_sb = cp.tile([1, L * 2 * H], f32, tag="ln1b")
            nc.sync.dma_start(
                ln1b_sb[:, :].rearrange("p (l m) -> p l m", l=L), ln1b_d.ap())
            b1_sb = cp.tile([1, L * 2 * H], f32, tag="b1")
            nc.sync.dma_start(
                b1_sb[:, :].rearrange("p (l m) -> p l m", l=L), b1_d.ap())
            b2_sb = cp.tile([1, L * H], f32, tag="b2")
            nc.sync.dma_start(
                b2_sb[:, :].rearrange("p (l m) -> p l m", l=L), b2_d.ap())
            encb_sb = cp.tile([1, H], f32, tag="encb")
            nc.sync.dma_start(encb_sb[:, :], encb_d.ap())
            linb_sb = cp.tile([1, C], f32, tag="linb")
            nc.sync.dma_start(linb_sb[:, :], linb_d.ap())
            t_sb = cp.tile([1, L], f32, tag="tv")
            nc.sync.dma_start(t_sb[:, :], t_d.ap())
            ident = cp.tile([128, 128], f32, tag="ident")
            make_identity(nc, ident[:, :])

            def freb(ap_1f, ntiles):
                """[1, F] AP -> [128, ntiles, F] (0-stride part & tile)."""
                b = ap_1f.partition_broadcast(128)      # [128, 1, F]
                b = b.broadcast_to(list(b.shape) + [ntiles])
                return b.rearrange("p a f t -> p (a t) f")

            def bias_const(val, tag):
                bt = cp.tile([128, 1], f32, tag=tag)
                nc.vector.memset(bt[:, :], val)
                return bt[:, :]

            b_exp = bias_const(EPS - M_OFF, "b_exp")
            b_ln = bias_const(1e-5, "b_ln")

            # ---- persistent node slabs ----
            h_sb = sp.tile([128, NF], f32, tag="h")
            z_sb = sp.tile([128, NF], f32, tag="z")
            pr_sb = sp.tile([128, TILES * 2 * H], bf16, tag="pr")
            scrA = sp.tile([128, NF], f32, tag="scrA")
            scrB = sp.tile([128, NF], f32, tag="scrB")

            # DRAM bounce + shared table (one per layer: Shared tensors
            # must have a single writer)
            pr_drams = []
            tables = []
            for l in range(L):
                prd_t = dp.tile([NPC, 2 * H], bf16, tag=f"prd{l}")
                tab_t = dp.tile([NPAD, 2 * H], bf16, tag=f"table{l}",
                                addr_space="Shared")
                pr_drams.append(prd_t)
                tables.append(tab_t)

            groups = [list(range(g, min(g + GROUP, TILES)))
                      for g in range(0, TILES, GROUP)]

            def h3():
                return h_sb[:, :].rearrange("p (t f) -> p t f", f=H)

            def z3():
                return z_sb[:, :].rearrange("p (t f) -> p t f", f=H)

            # ============== ENCODER: h = x @ encW + encb ==============
            with tc.tile_pool(name="enc", bufs=1) as ep:
                x_sb = ep.tile([128, TILES * F_IN], f32, tag="xslab")
                nc.sync.dma_start(
                    x_sb[:, :].rearrange("p (t f) -> p t f", f=F_IN),
                    x_d.ap().rearrange("(t p) f -> p t f", p=128))
                for tiles in groups:
                    ps_h = pp.tile([128, GROUP * H], f32, tag="y2")
                    for i, t in enumerate(tiles):
                        ps_t = pp.tile([128, 128], f32, tag="tr")
                        nc.tensor.transpose(
                            out=ps_t[:, :],
                            in_=x_sb[:, t * F_IN:(t + 1) * F_IN],
                            identity=ident[:, :])
                        xT = wp.tile([128, 128], f32, tag="lhs")
                        nc.vector.tensor_copy(out=xT[:, :], in_=ps_t[:, :])
                        nc.tensor.matmul(
                            out=ps_h[:, i * H:(i + 1) * H],
                            lhsT=xT[:, :], rhs=encw_sb[:, :],
                            start=True, stop=True)
                    sl = slice(tiles[0] * H, (tiles[-1] + 1) * H)
                    if encb_triv:
                        nc.vector.tensor_copy(
                            out=h_sb[:, sl], in_=ps_h[:, :len(tiles) * H])
                    else:
                        nc.vector.tensor_tensor(
                            out=h_sb[:, sl].rearrange("p (t f) -> p t f", f=H),
                            in0=ps_h[:, :len(tiles) * H].rearrange(
                                "p (t f) -> p t f", f=H),
                            in1=encb_sb[0:1, :].partition_broadcast(128)
                                .broadcast_to([128, 1, H, len(tiles)])
                                .rearrange("p a f t -> p (a t) f"),
                            op=OP.add)

            # ============== LAYERS ==============
            for l in range(nlayers):
                # ----- node phase: z (pre-norm), V, P, R -----
                if l == 0:
                    zsrc = h_sb  # conv input is encoder output directly
                    # V = relu(z) needed (z may be negative)
                    nc.scalar.activation(
                        out=scrA[:, :], in_=h_sb[:, :], func=AF.Relu)
                    vsrc = scrA
                else:
                    # z = relu(LN(h; g, b)) ; V = z (already >= 0)
                    mu = wp.tile([128, TILES], f32, tag="mu")
                    nc.vector.reduce_sum(out=mu[:, :], in_=h3(), axis=AX.X)
                    nc.vector.tensor_scalar(
                        out=mu[:, :], in0=mu[:, :], scalar1=1.0 / H,
                        scalar2=None, op0=OP.mult)
                    cent3 = scrA[:, :].rearrange("p (t f) -> p t f", f=H)
                    nc.vector.tensor_tensor(
                        out=cent3, in0=h3(),
                        in1=mu[:, :].broadcast_to([128, TILES, H]),
                        op=OP.subtract)
                    nc.scalar.activation(
                        out=scrB[:, :], in_=scrA[:, :], func=AF.Square)
                    var = wp.tile([128, TILES], f32, tag="var")
                    nc.vector.reduce_sum(
                        out=var[:, :],
                        in_=scrB[:, :].rearrange("p (t f) -> p t f", f=H),
                        axis=AX.X)
                    nc.scalar.activation(
                        out=var[:, :], in_=var[:, :], func=AF.Sqrt,
                        bias=b_ln, scale=1.0 / H)
                    rs = wp.tile([128, TILES], f32, tag="rs")
                    nc.vector.reciprocal(out=rs[:, :], in_=var[:, :])
                    nc.vector.tensor_tensor(
                        out=scrB[:, :].rearrange("p (t f) -> p t f", f=H),
                        in0=cent3,
                        in1=rs[:, :].broadcast_to([128, TILES, H]),
                        op=OP.mult)
                    # apply norm_g / norm_b if non-trivial
                    if not triv["norm"]:
                        sb3 = scrB[:, :].rearrange("p (t f) -> p t f", f=H)
                        nc.vector.tensor_tensor(
                            out=sb3, in0=sb3,
                            in1=freb(ngrep_sb[0:1, l * H:(l + 1) * H], TILES),
                            op=OP.mult)
                        nc.vector.tensor_tensor(
                            out=sb3, in0=sb3,
                            in1=freb(nbrep_sb[0:1, l * H:(l + 1) * H], TILES),
                            op=OP.add)
                    nc.scalar.activation(
                        out=z_sb[:, :], in_=scrB[:, :], func=AF.Relu)
                    vsrc = z_sb
                z_cur = h_sb if l == 0 else z_sb

                # P = exp(t*(V+eps) - 8) -> pr[:, :, 0:H] (bf16)
                pr3 = pr_sb[:, :].rearrange("p (t f) -> p t f", f=2 * H)
                v3 = vsrc[:, :].rearrange("p (t f) -> p t f", f=H)
                if t_triv:
                    nc.scalar.activation(
                        out=pr3[:, :, 0:H], in_=v3, func=AF.Exp,
                        bias=b_exp, scale=1.0)
                else:
                    tb = wp.tile([1, 1], f32, tag="tb")
                    nc.vector.tensor_scalar(
                        out=tb[0:1, 0:1], in0=t_sb[0:1, l:l + 1],
                        scalar1=EPS, scalar2=-M_OFF, op0=OP.mult, op1=OP.add)
                    nc.scalar.activation(
                        out=pr3[:, :, 0:H], in_=v3, func=AF.Exp,
                        bias=pb(tb[0:1, 0:1]), scale=pb(t_sb[0:1, l:l + 1]))
                # R = (V+eps)*P -> pr[:, :, H:2H]
                nc.vector.scalar_tensor_tensor(
                    out=pr3[:, :, H:2 * H], in0=v3, scalar=EPS,
                    in1=pr3[:, :, 0:H], op0=OP.add, op1=OP.mult)

                # table write + AllGather
                pr_dram = pr_drams[l]
                table = tables[l]
                nc.sync.dma_start(
                    pr_dram[:, :].rearrange("(t p) f -> p t f", p=128),
                    pr3)
                if stage == "nocc":
                    nc.sync.dma_start(table[0:NPC, :], pr_dram[:, :])
                else:
                    nc.gpsimd.collective_compute(
                        "AllGather", OP.bypass,
                        replica_groups=[list(range(NC_))],
                        ins=[pr_dram.opt()], outs=[table.opt()])

                # ----- edge phase + MLP, grouped by PSUM bank -----
                for tiles in groups:
                    ng = len(tiles)
                    ps_e = pp.tile([128, GROUP * 2 * H], f32, tag="edge")
                    for i, t in enumerate(tiles):
                        G = gp.tile([128, MAXB * 128], bf16, tag="G")
                        nb = int(n_tile[t]) // 128
                        G3 = G[:, :nb * 128].rearrange(
                            "p (j f) -> p j f", f=128)
                        if stage in ("gather", "full"):
                            nc.gpsimd.dma_gather(
                                out_ap=G3,
                                in_ap=table[:, :],
                                idxs_ap=idx_sb[:, int(tcb[t]):int(tcb[t]) + int(n_tile[t]) // 16],
                                num_idxs=int(n_tile[t]),
                                num_idxs_reg=int(n_tile[t]),
                                elem_size=2 * H,
                                single_packet=False)
                        if stage != "full":
                            nc.vector.memset(
                                ps_e[:, i * 2 * H:(i + 1) * 2 * H], 1.0)
                            continue
                        for (j, w, st, sp_) in batches[t]:
                            nc.tensor.matmul(
                                out=ps_e[w * WIN:(w + 1) * WIN,
                                         i * 2 * H:(i + 1) * 2 * H],
                                lhsT=oneh_sb[:, (int(tbb[t]) + j) * WIN:
                                             (int(tbb[t]) + j + 1) * WIN],
                                rhs=G3[:, j, :],
                                start=st, stop=sp_,
                                tile_position=(0, w * WIN))
                    # agg = numer/(denom+1e-16) + z  (batched over group)
                    pe3 = ps_e[:, :ng * 2 * H].rearrange(
                        "p (t f) -> p t f", f=2 * H)
                    den = grp_pool.tile([128, GROUP * H], f32, tag="den")
                    den3 = den[:, :ng * H].rearrange("p (t f) -> p t f", f=H)
                    nc.vector.tensor_scalar(
                        out=den3, in0=pe3[:, :, 0:H], scalar1=1e-16,
                        scalar2=None, op0=OP.add)
                    rec = grp_pool.tile([128, GROUP * H], f32, tag="rec")
                    nc.vector.reciprocal(
                        out=rec[:, :ng * H], in_=den[:, :ng * H])
                    mlpin = grp_pool.tile([128, GROUP * H], f32, tag="mlpin")
                    mi3 = mlpin[:, :ng * H].rearrange("p (t f) -> p t f", f=H)
                    nc.vector.tensor_tensor(
                        out=mi3, in0=pe3[:, :, H:2 * H],
                        in1=rec[:, :ng * H].rearrange("p (t f) -> p t f", f=H),
                        op=OP.mult)
                    zsl = slice(tiles[0] * H, (tiles[-1] + 1) * H)
                    nc.vector.tensor_tensor(
                        out=mi3, in0=mi3,
                        in1=z_cur[:, zsl].rearrange("p (t f) -> p t f", f=H),
                        op=OP.add)

                    # --- MLP part 1: y1 = mlpin @ W1 (per tile) ---
                    ps_y1 = pp.tile([128, GROUP * 2 * H], f32, tag="y1")
                    for i, t in enumerate(tiles):
                        ps_t = pp.tile([128, 128], f32, tag="tr")
                        nc.tensor.transpose(
                            out=ps_t[:H, :],
                            in_=mlpin[:, i * H:(i + 1) * H],
                            identity=ident[:, :])
                        mT = wp.tile([128, 128], f32, tag="lhs")
                        nc.vector.tensor_copy(
                            out=mT[:H, :], in_=ps_t[:H, :])
                        nc.tensor.matmul(
                            out=ps_y1[:, i * 2 * H:(i + 1) * 2 * H],
                            lhsT=mT[:H, :],
                            rhs=w1_sb[:, l * 2 * H:(l + 1) * 2 * H],
                            start=True, stop=True)
                    # --- LN1 + relu (batched over group) ---
                    py3 = ps_y1[:, :ng * 2 * H].rearrange(
                        "p (t f) -> p t f", f=2 * H)
                    cent = grp_pool.tile([128, GROUP * 2 * H], f32, tag="cent")
                    c3 = cent[:, :ng * 2 * H].rearrange(
                        "p (t f) -> p t f", f=2 * H)
                    if not b1_triv:
                        nc.vector.tensor_tensor(
                            out=py3, in0=py3,
                            in1=b1_sb[0:1, l * 2 * H:(l + 1) * 2 * H]
                                .partition_broadcast(128)
                                .broadcast_to([128, 1, 2 * H, ng])
                                .rearrange("p a f t -> p (a t) f"),
                            op=OP.add)
                    mu1 = wp.tile([128, GROUP], f32, tag="mu1")
                    nc.vector.reduce_sum(
                        out=mu1[:, :ng], in_=py3, axis=AX.X)
                    nc.vector.tensor_scalar(
                        out=mu1[:, :ng], in0=mu1[:, :ng],
                        scalar1=1.0 / (2 * H), scalar2=None, op0=OP.mult)
                    nc.vector.tensor_tensor(
                        out=c3, in0=py3,
                        in1=mu1[:, :ng].broadcast_to([128, ng, 2 * H]),
                        op=OP.subtract)
                    sq = grp_pool.tile([128, GROUP * 2 * H], f32, tag="sq")
                    nc.scalar.activation(
                        out=sq[:, :ng * 2 * H], in_=cent[:, :ng * 2 * H],
                        func=AF.Square)
                    v1 = wp.tile([128, GROUP], f32, tag="v1")
                    nc.vector.reduce_sum(
                        out=v1[:, :ng],
                        in_=sq[:, :ng * 2 * H].rearrange(
                            "p (t f) -> p t f", f=2 * H),
                        axis=AX.X)
                    nc.scalar.activation(
                        out=v1[:, :ng], in_=v1[:, :ng], func=AF.Sqrt,
                        bias=b_ln, scale=1.0 / (2 * H))
                    rs1 = wp.tile([128, GROUP], f32, tag="rs1")
                    nc.vector.reciprocal(out=rs1[:, :ng], in_=v1[:, :ng])
                    z2 = grp_pool.tile([128, GROUP * 2 * H], f32, tag="z2")
                    z23 = z2[:, :ng * 2 * H].rearrange(
                        "p (t f) -> p t f", f=2 * H)
                    nc.vector.tensor_tensor(
                        out=z23, in0=c3,
                        in1=rs1[:, :ng].broadcast_to([128, ng, 2 * H]),
                        op=OP.mult)
                    if not ln1_triv:
                        gb_ap = ln1g_sb[0:1, l * 2 * H:(l + 1) * 2 * H] \
                            .partition_broadcast(128) \
                            .broadcast_to([128, 1, 2 * H, ng]) \
                            .rearrange("p a f t -> p (a t) f")
                        nc.vector.tensor_tensor(
                            out=z23, in0=z23, in1=gb_ap, op=OP.mult)
                        bb_ap = ln1b_sb[0:1, l * 2 * H:(l + 1) * 2 * H] \
                            .partition_broadcast(128) \
                            .broadcast_to([128, 1, 2 * H, ng]) \
                            .rearrange("p a f t -> p (a t) f")
                        nc.vector.tensor_tensor(
                            out=z23, in0=z23, in1=bb_ap, op=OP.add)
                    nc.scalar.activation(
                        out=z2[:, :ng * 2 * H], in_=z2[:, :ng * 2 * H],
                        func=AF.Relu)
                    # --- MLP part 2: y2 = z2 @ W2 ; h update ---
                    ps_y2 = pp.tile([128, GROUP * H], f32, tag="y2")
                    for i, t in enumerate(tiles):
                        ps_t2 = pp.tile([128, 128], f32, tag="tr")
                        nc.tensor.transpose(
                            out=ps_t2[:, :],
                            in_=z2[:, i * 2 * H:(i + 1) * 2 * H],
                            identity=ident[:, :])
                        zT = wp.tile([128, 128], f32, tag="lhs")
                        nc.vector.tensor_copy(out=zT[:, :], in_=ps_t2[:, :])
                        nc.tensor.matmul(
                            out=ps_y2[:, i * H:(i + 1) * H],
                            lhsT=zT[:, :],
                            rhs=w2_sb[:, l * H:(l + 1) * H],
                            start=True, stop=True)
                    py2_3 = ps_y2[:, :ng * H].rearrange(
                        "p (t f) -> p t f", f=H)
                    hsl = slice(tiles[0] * H, (tiles[-1] + 1) * H)
                    if not b2_triv:
                        nc.vector.tensor_tensor(
                            out=py2_3, in0=py2_3,
                            in1=b2_sb[0:1, l * H:(l + 1) * H]
                                .partition_broadcast(128)
                                .broadcast_to([128, 1, H, ng])
                                .rearrange("p a f t -> p (a t) f"),
                            op=OP.add)
                    if l == 0:
                        nc.vector.tensor_copy(
                            out=h_sb[:, hsl], in_=ps_y2[:, :ng * H])
                    else:
                        nc.vector.tensor_tensor(
                            out=h_sb[:, hsl], in0=ps_y2[:, :ng * H],
                            in1=h_sb[:, hsl], op=OP.add)

            # ============== FINAL: relu(LN(h; g0, b0)) @ linW -> lsm ==========
            mu = wp.tile([128, TILES], f32, tag="mu")
            nc.vector.reduce_sum(out=mu[:, :], in_=h3(), axis=AX.X)
            nc.vector.tensor_scalar(
                out=mu[:, :], in0=mu[:, :], scalar1=1.0 / H, scalar2=None,
                op0=OP.mult)
            cent3 = scrA[:, :].rearrange("p (t f) -> p t f", f=H)
            nc.vector.tensor_tensor(
                out=cent3, in0=h3(),
                in1=mu[:, :].broadcast_to([128, TILES, H]), op=OP.subtract)
            nc.scalar.activation(
                out=scrB[:, :], in_=scrA[:, :], func=AF.Square)
            var = wp.tile([128, TILES], f32, tag="var")
            nc.vector.reduce_sum(
                out=var[:, :],
                in_=scrB[:, :].rearrange("p (t f) -> p t f", f=H), axis=AX.X)
            nc.scalar.activation(
                out=var[:, :], in_=var[:, :], func=AF.Sqrt,
                bias=b_ln, scale=1.0 / H)
            rs = wp.tile([128, TILES], f32, tag="rs")
            nc.vector.reciprocal(out=rs[:, :], in_=var[:, :])
            nc.vector.tensor_tensor(
                out=scrB[:, :].rearrange("p (t f) -> p t f", f=H),
                in0=cent3, in1=rs[:, :].broadcast_to([128, TILES, H]),
                op=OP.mult)
            if not triv["norm"]:
                sb3f = scrB[:, :].rearrange("p (t f) -> p t f", f=H)
                nc.vector.tensor_tensor(
                    out=sb3f, in0=sb3f,
                    in1=freb(ngrep_sb[0:1, 0:H], TILES), op=OP.mult)
                nc.vector.tensor_tensor(
                    out=sb3f, in0=sb3f,
                    in1=freb(nbrep_sb[0:1, 0:H], TILES), op=OP.add)
            nc.scalar.activation(
                out=scrA[:, :], in_=scrB[:, :], func=AF.Relu)
            # logits per tile
            lg = sp.tile([128, TILES * C], f32, tag="lg")
            for tiles in groups:
                ps_lg = pp.tile([128, GROUP * H], f32, tag="y2")
                for i, t in enumerate(tiles):
                    ps_t = pp.tile([128, 128], f32, tag="tr")
                    nc.tensor.transpose(
                        out=ps_t[:H, :],
                        in_=scrA[:, t * H:(t + 1) * H],
                        identity=ident[:, :])
                    fT = wp.tile([128, 128], f32, tag="lhs")
                    nc.vector.tensor_copy(out=fT[:H, :], in_=ps_t[:H, :])
                    nc.tensor.matmul(
                        out=ps_lg[:, i * H:i * H + C],
                        lhsT=fT[:H, :], rhs=linw_sb[:, :],
                        start=True, stop=True)
                for i, t in enumerate(tiles):
                    if linb_triv:
                        nc.vector.tensor_copy(
                            out=lg[:, t * C:(t + 1) * C],
                            in_=ps_lg[:, i * H:i * H + C])
                    else:
                        nc.vector.tensor_tensor(
                            out=lg[:, t * C:(t + 1) * C],
                            in0=ps_lg[:, i * H:i * H + C],
                            in1=pb(linb_sb[0:1, :]), op=OP.add)
            # batched log_softmax over C
            lg3 = lg[:, :].rearrange("p (t c) -> p t c", c=C)
            mx = wp.tile([128, TILES], f32, tag="mx")
            nc.vector.reduce_max(out=mx[:, :], in_=lg3, axis=AX.X)
            sh = scrA
            sh3 = sh[:, :TILES * C].rearrange("p (t c) -> p t c", c=C)
            nc.vector.tensor_tensor(
                out=sh3, in0=lg3,
                in1=mx[:, :].broadcast_to([128, TILES, C]), op=OP.subtract)
            ex = scrB
            nc.scalar.activation(out=ex[:, :TILES * C], in_=sh[:, :TILES * C],
                                 func=AF.Exp)
            sm = wp.tile([128, TILES], f32, tag="sm")
            nc.vector.reduce_sum(
                out=sm[:, :],
                in_=ex[:, :TILES * C].rearrange("p (t c) -> p t c", c=C),
                axis=AX.X)
            nc.scalar.activation(out=sm[:, :], in_=sm[:, :], func=AF.Ln)
            nc.vector.tensor_tensor(
                out=sh3, in0=sh3,
                in1=sm[:, :].broadcast_to([128, TILES, C]), op=OP.subtract)
            nc.sync.dma_start(
                out_d.ap().rearrange("(t p) c -> p t c", p=128), sh3)

    nc.compile()
    return nc


# --------------------------------------------------------------------------
# Entry point
# --------------------------------------------------------------------------

def kernel(x, edge_index, enc_W, enc_b, t, W1, b1, ln1_g, ln1_b, W2, b2,
           norm_g, norm_b, lin_W, lin_b):
    global LAST_RESULTS
    from concourse.bass_utils import run_bass_kernel_spmd

    x = np.ascontiguousarray(np.asarray(x, dtype=np.float32))
    edge_index = np.asarray(edge_index)
    key = hash((edge_index.tobytes(),))

    triv = dict(
        t=bool(np.allclose(np.asarray(t), 1.0)),
        ln1=bool(np.allclose(np.asarray(ln1_g), 1.0)
                 and np.allclose(np.asarray(ln1_b), 0.0)),
        b1=bool(np.allclose(np.asarray(b1), 0.0)),
        b2=bool(np.allclose(np.asarray(b2), 0.0)),
        encb=bool(np.allclose(np.asarray(enc_b), 0.0)),
        linb=bool(np.allclose(np.asarray(lin_b), 0.0)),
        norm=bool(np.allclose(np.asarray(norm_g), 1.0)
                  and np.allclose(np.asarray(norm_b), 0.0)),
    )
    ckey = (key, tuple(sorted(triv.items())))
    if ckey in _CACHE:
        meta, nc = _CACHE[ckey]
    else:
        meta = _preprocess(edge_index)
        nc = _build(meta, triv)
        _CACHE.clear()
        _CACHE[ckey] = (meta, nc)

    f32c = lambda a: np.ascontiguousarray(np.asarray(a, dtype=np.float32))
    node_of = meta["node_of"]
    L2H = 2 * H

    shared = dict(
        encW=f32c(enc_W),
        encb=f32c(enc_b).reshape(1, H),
        tvec=f32c(t).reshape(1, L),
        w1=f32c(np.transpose(np.asarray(W1), (1, 0, 2))),      # [H, L, 2H]
        b1r=f32c(b1).reshape(1, L, L2H),
        ln1g=f32c(ln1_g).reshape(1, L, L2H),
        ln1b=f32c(ln1_b).reshape(1, L, L2H),
        w2=f32c(np.transpose(np.asarray(W2), (1, 0, 2))),      # [2H, L, H]
        b2r=f32c(b2).reshape(1, L, H),
        ngrep=f32c(norm_g).reshape(1, L, H),
        nbrep=f32c(norm_b).reshape(1, L, H),
        linW=f32c(lin_W),
        linb=f32c(lin_b).reshape(1, C),
    )

    in_maps = []
    for c in range(NC_):
        xs = np.zeros((NPC, F_IN), np.float32)
        valid = node_of[c] >= 0
        xs[valid] = x[node_of[c][valid]]
        m = dict(shared)
        m["x_sh"] = xs
        m["idxs"] = np.ascontiguousarray(meta["idx_slab"][c])
        m["oneh"] = np.ascontiguousarray(meta["oneh"][c])
        in_maps.append(m)

    res = run_bass_kernel_spmd(nc, in_maps, core_ids=list(range(NC_)))
    LAST_RESULTS = res

    out = np.empty((N, C), np.float32)
    for c in range(NC_):
        o = res.results[c]["out"]
        valid = node_of[c] >= 0
        out[node_of[c][valid]] = o[valid]
    return out
